# revision 18
# baseline (speedup 1.0000x reference)
"""Trainium2 Bass kernel for nn_BilinearEquivariantLayer (v4, pair-sharded).

Per core c of 8, SPMD. Core c's own columns: cols(c) = {32c..32c+32} u
{256+32c..256+32c+32} (host V permutation). Pair partner p = c^4; group
G = c//4 (cores 0-3 / 4-7).

  st1: A_pos^T[k] = (P[k] @ V[k])^T for k in {2c, 2c+1}; each 32-row
       strip is routed to TWO dests (owner and owner^4); two AllToAlls
       redistribute so core c holds all k for l-set = cols(c)+cols(c^4)
       (128 cols; l = q*64 + blk*32 + sw, q=0 own / q=1 partner).
  st2: irfft over k as matmul vs block-diag CIR -> A_real in SBUF
       [D-chunk p, dc4, t32, l128] (bf16).
  st3: per t8-group tg: W1A[t,h,d,own 64] -> grouped AllGather
       (replica groups [0-3],[4-7], 4 chunks); W2A[t,h,d,l128] -> SBUF.
  st4: bilinear U[t,h][l128, s~256] = W2A^T @ W1A_group per (t, head),
       full m=128 matmuls; s~ = (gi4, c64) = group G's columns. U
       staged to DRAM u_dr[cc, (h4 t), l, s~] via one 512KB DMA per
       (tg, h).
  st5: fused rfft+mixer as matmul G^T @ U, streaming u_dr in 512KB
       slices; output planes o[mc, (k j), (l128, s~256)].
Host assembles complex64 output.
"""
import sys
sys.path.insert(0, "/opt/trn_rl_repo")
import os
import numpy as np
from concourse import bass, bacc, tile, mybir
from concourse import bass_utils

NCORES = 8
K, D, N, R, H, dproj = 16, 512, 1024, 512, 8, 128
T = 2 * K - 1           # 31
KL = K // NCORES        # 2 k's per core
LC = 128                # l-set size (own + partner columns)
F32 = mybir.dt.float32
BF16 = mybir.dt.bfloat16

# precision mode: "e3m4" stores U in fp8_e3m4 (halves st4-write/st5-read
# DMA); "bf16" is the safe fallback.  W1A stays bf16 either way.
PREC = os.environ.get("KPREC", "e3m4")
W1A_DT = BF16
W1_SCALE = 1.0
if PREC == "e3m4":
    U_DT = mybir.dt.float8e3
    W2_SCALE = 2.0 ** -18
    G_SCALE = 2.0 ** 18
else:
    U_DT = BF16
    W2_SCALE = 1.0
    G_SCALE = 1.0

_CACHE = {}


def _copy(nc, i, out, in_):
    """Rotate psum->sbuf copies across vector/scalar (gpsimd can't read
    PSUM)."""
    if i % 2 == 0:
        nc.vector.tensor_copy(out, in_)
    else:
        nc.scalar.copy(out, in_)


def _build():
    nc = bacc.Bacc("TRN2", target_bir_lowering=False, debug=False,
                   num_devices=NCORES)
    pt = nc.dram_tensor("pt", [KL, 2, N, D], BF16, kind="ExternalInput").ap()
    v = nc.dram_tensor("v", [KL, N, R], BF16, kind="ExternalInput").ap()
    w1t = nc.dram_tensor("w1t", [D, H * dproj], BF16,
                         kind="ExternalInput").ap()
    w2t = nc.dram_tensor("w2t", [D, H * dproj], BF16,
                         kind="ExternalInput").ap()
    cirb = nc.dram_tensor("cirb", [128, 128], BF16, kind="ExternalInput").ap()
    g = nc.dram_tensor("g", [2, 128, 256], BF16, kind="ExternalInput").ap()
    # out planes: [m(re/im), (k j), (l128, s~256)]
    o = nc.dram_tensor("o", [2, 128, LC * 256], BF16,
                       kind="ExternalOutput").ap()

    with tile.TileContext(nc) as tc:
        with tc.tile_pool(name="dram", bufs=1, space="DRAM") as dram:
            # A2A buffers: [dest, kl, l-slot 128, ri, D]
            a2a_in = dram.tile([NCORES, KL, LC, 2, D], BF16, name="a2ain")
            a2a_out = dram.tile([NCORES, KL, LC, 2, D], BF16, name="a2aout")
            # W1A grouped AllGather (one op): [h, d, t32, c64]
            ag_in = dram.tile([H, dproj, 32, 64], W1A_DT, name="agin")
            w1ag = dram.tile([4, H, dproj, 32, 64], W1A_DT, name="w1ag")
            # U staging: [cc, (h4 t)=128, l128, s~256]
            u_dr = dram.tile([2, 128, LC, 256], U_DT, name="udr")

            with tc.tile_pool(name="big", bufs=1) as big:
                w2a_sb = big.tile([dproj, H, 32, LC], BF16)   # 8.4MB
                g_sb = big.tile([128, 2, 256], BF16)
                nc.scalar.dma_start(out=g_sb[:], in_=g.rearrange(
                    "a p b -> p a b"))

                # ---- stage 1: A_pos^T -> a2a_in (strips to 2 dests)
                sc1 = nc.named_scope("st1"); sc1.__enter__()
                with tc.tile_pool(name="s1", bufs=1) as s1, \
                     tc.tile_pool(name="s1c", bufs=4) as s1c, \
                     tc.tile_pool(name="ps1", bufs=3, space="PSUM") as ps1p:
                    pt_sb = s1.tile([128, KL, 2, 8, D], BF16)
                    v_sb = s1.tile([128, KL, 8, R], BF16)
                    for kl in range(KL):
                        eng = nc.sync if kl == 0 else nc.scalar
                        eng.dma_start(
                            out=v_sb[:, kl, :, :],
                            in_=v[kl].rearrange("(a p) d -> p a d", p=128))
                        for ri in range(2):
                            eng = nc.sync if ri == 0 else nc.scalar
                            eng.dma_start(
                                out=pt_sb[:, kl, ri, :, :],
                                in_=pt[kl, ri].rearrange(
                                    "(a p) d -> p a d", p=128))
                    for kl in range(KL):
                        for ri in range(2):
                            for rcc in range(4):
                                # psum = A_pos^T chunk [rc 128, D 512]
                                ps1 = ps1p.tile([128, D], F32, tag="ps1")
                                for nci in range(8):
                                    nc.tensor.matmul(
                                        ps1[:],
                                        v_sb[:, kl, nci,
                                             rcc * 128:(rcc + 1) * 128],
                                        pt_sb[:, kl, ri, nci, :],
                                        start=(nci == 0), stop=(nci == 7))
                                cp1 = s1c.tile([128, D], BF16, tag="cp1")
                                _copy(nc, rcc, cp1[:], ps1[:])
                                # rows = cores {2rcc, 2rcc+1} x 64; send to
                                # owner (q=0 slots) and partner (q=1 slots)
                                qd = (2 * rcc + 4) % 8
                                deng = nc.sync if rcc % 2 == 0 else nc.scalar
                                deng.dma_start(
                                    out=a2a_in[2 * rcc:2 * rcc + 2, kl,
                                               0:64, ri, :],
                                    in_=cp1[:])
                                deng2 = nc.scalar if rcc % 2 == 0 else nc.sync
                                deng2.dma_start(
                                    out=a2a_in[qd:qd + 2, kl,
                                               64:128, ri, :],
                                    in_=cp1[:])
                    nc.gpsimd.collective_compute(
                        "AllToAll", mybir.AluOpType.bypass,
                        replica_groups=[list(range(NCORES))],
                        ins=[a2a_in.opt()],
                        outs=[a2a_out.opt()])
                sc1.__exit__(None, None, None)

                # ---- stage 2: irfft -> ar_sb [128 Dp, dc4, t32, l128]
                sc2 = nc.named_scope("st2"); sc2.__enter__()
                with tc.tile_pool(name="mid", bufs=1) as mid, \
                     tc.tile_pool(name="a2asb", bufs=4) as a2ap, \
                     tc.tile_pool(name="ps2", bufs=4, space="PSUM") as ps2p:
                    ar_sb = mid.tile([128, 4, 32, LC], BF16)
                    w1t_sb = mid.tile([128, 4, H * dproj], BF16)
                    w2t_sb = mid.tile([128, 4, H * dproj], BF16)
                    cirb_sb = mid.tile([128, 128], BF16)
                    nc.scalar.dma_start(out=cirb_sb[:], in_=cirb[:, :])
                    nc.scalar.dma_start(
                        out=w1t_sb[:],
                        in_=w1t.rearrange("(a p) f -> p a f", p=128))
                    nc.scalar.dma_start(
                        out=w2t_sb[:],
                        in_=w2t.rearrange("(a p) f -> p a f", p=128))

                    # a2aq partitions = (kl, src, r4, ri); free = D.
                    ps2l = {}
                    for rq in range(32):
                        a2aq = a2ap.tile([128, D], BF16, tag="a2aq",
                                         name="a2aq")
                        eng = nc.sync if rq % 2 == 0 else nc.scalar
                        eng.dma_start(
                            out=a2aq[:],
                            in_=a2a_out[:, :, rq * 4:(rq + 1) * 4, :,
                                        :].rearrange(
                                "s a r k d -> (s a) (r k) d"))
                        rqg, j = rq // 4, rq % 4
                        for dc in range(4):
                            if j == 0:
                                ps2l[dc] = ps2p.tile([128, 512], F32,
                                                     tag="ps2",
                                                     name=f"ps2_{dc}")
                            nc.tensor.matmul(
                                ps2l[dc][:, j * 128:(j + 1) * 128],
                                a2aq[:, dc * 128:(dc + 1) * 128],
                                cirb_sb[:],
                                start=True, stop=True)
                        if j != 3:
                            continue
                        for dc in range(4):
                            _copy(nc, dc,
                                  ar_sb[:, dc, :,
                                        rqg * 16:rqg * 16 + 16],
                                  ps2l[dc][:].rearrange(
                                      "p (j r t) -> p t (j r)",
                                      j=4, r=4))
                    sc2.__exit__(None, None, None)

                    # ---- stage 3: W-projections per t8-group
                    sc3 = nc.named_scope("st3"); sc3.__enter__()
                    with tc.tile_pool(name="agst", bufs=2) as agstp, \
                         tc.tile_pool(name="ps3", bufs=3, space="PSUM") \
                            as ps3p:
                        def proj(wsb, tg, h, l0):
                            ps3 = ps3p.tile([128, 512], F32, tag="ps3")
                            for dc in range(4):
                                nc.tensor.matmul(
                                    ps3[:],
                                    wsb[:, dc, h * 128:(h + 1) * 128],
                                    ar_sb[:, dc, tg * 8:(tg + 1) * 8,
                                          l0:l0 + 64],
                                    start=(dc == 0), stop=(dc == 3))
                            return ps3

                        for tg in range(4):
                            stg = agstp.tile([128, H, 512], W1A_DT,
                                             tag="stg")
                            for h in range(H):
                                ps3 = proj(w1t_sb, tg, h, 0)
                                _copy(nc, h, stg[:, h, :], ps3[:])
                            # one DMA: [d p, (h,t8,c)] -> [h,d,t8,c]
                            nc.gpsimd.dma_start(
                                out=ag_in.transpose(
                                    [1, 0, 2, 3])[:, :,
                                                  tg * 8:(tg + 1) * 8,
                                                  :],
                                in_=stg[:])
                        nc.gpsimd.collective_compute(
                            "AllGather", mybir.AluOpType.bypass,
                            replica_groups=[[0, 1, 2, 3],
                                            [4, 5, 6, 7]],
                            ins=[ag_in.opt()],
                            outs=[w1ag.opt()])
                        for tg in range(4):
                            for h in range(H):
                                for lh in range(2):
                                    ps3 = proj(w2t_sb, tg, h, lh * 64)
                                    _copy(nc, h + lh,
                                          w2a_sb[:, h,
                                                 tg * 8:(tg + 1) * 8,
                                                 lh * 64:(lh + 1) * 64],
                                          ps3[:].rearrange(
                                              "p (t c) -> p t c", t=8))
                    sc3.__exit__(None, None, None)

                # ---- stage 4: bilinear per (tg, head), m=128
                sc4 = nc.named_scope("st4"); sc4.__enter__()
                with tc.tile_pool(name="w1x", bufs=6) as w1xp, \
                     tc.tile_pool(name="ust", bufs=3) as ustp, \
                     tc.tile_pool(name="ps4", bufs=3, space="PSUM") as ps4p:
                    def w1x_load(s):
                        tg, h = s // 8, s % 8
                        w1xt = w1xp.tile([dproj, 4, 8, 64], W1A_DT,
                                         tag="w1x", name="w1x")
                        eng = nc.sync if s % 2 == 0 else nc.scalar
                        # [d p, (gi4, t8, c64)] <- w1ag[gi, h, :, tg-t8]
                        eng.dma_start(
                            out=w1xt[:],
                            in_=w1ag[:, h, :, tg * 8:(tg + 1) * 8,
                                     :].transpose([1, 0, 2, 3]))
                        return w1xt

                    pending = [w1x_load(0), w1x_load(1), w1x_load(2)]
                    for s in range(32):
                        tg, h = s // 8, s % 8
                        cc, h4 = h // 4, h % 4
                        cur = pending.pop(0)
                        if s + 3 < 32:
                            pending.append(w1x_load(s + 3))
                        ust = ustp.tile([128, 8, 256], U_DT, tag="ust")
                        for tp in range(4):
                            ps4 = ps4p.tile([128, 512], F32, tag="ps4")
                            for ti in range(2):
                                tl = tp * 2 + ti
                                t = tg * 8 + tl
                                nc.tensor.matmul(
                                    ps4[:, ti * 256:(ti + 1) * 256],
                                    w2a_sb[:, h, t, :],
                                    cur[:, :, tl, :],
                                    start=True, stop=True)
                            _copy(nc, tp,
                                  ust[:, tp * 2:tp * 2 + 2, :],
                                  ps4[:].rearrange("p (a b) -> p a b",
                                                   a=2))
                        rb = h4 * 32 + tg * 8
                        deng = nc.sync if s % 2 == 0 else nc.scalar
                        deng.dma_start(
                            out=u_dr[cc, rb:rb + 8, :, :].transpose(
                                [1, 0, 2]),
                            in_=ust[:])
                sc4.__exit__(None, None, None)

                # ---- stage 5: fused rfft + mixer, streaming u slices
                sc5 = nc.named_scope("st5"); sc5.__enter__()
                with tc.tile_pool(name="u5", bufs=6) as u5p, \
                     tc.tile_pool(name="ost", bufs=3) as ostp, \
                     tc.tile_pool(name="ps5", bufs=4, space="PSUM") as ps5p:
                    def u_load(fb):
                        pair = []
                        for cci in range(2):
                            ut = u5p.tile([128, 16, 256], U_DT, tag="u5",
                                          name="u5")
                            nc.sync.dma_start(
                                out=ut[:],
                                in_=u_dr[cci, :, fb * 16:(fb + 1) * 16,
                                         :])
                            pair.append(ut)
                        return pair

                    upend = [u_load(0), u_load(1)]
                    for fb in range(8):
                        uc = upend.pop(0)
                        if fb + 2 < 8:
                            upend.append(u_load(fb + 2))
                        osts = [ostp.tile([128, 4096], BF16, tag="ost",
                                           name="ost")
                                for _ in range(2)]
                        for fl in range(2):
                            for mc in range(2):
                                for f in range(4):
                                    fi = fl * 4 + f
                                    ps5 = ps5p.tile([128, 512], F32,
                                                    tag="ps5")
                                    for cci in range(2):
                                        nc.tensor.matmul(
                                            ps5[:],
                                            g_sb[:, cci,
                                                 mc * 128:
                                                 (mc + 1) * 128],
                                            uc[cci][:, fi * 2:fi * 2 + 2,
                                                    :],
                                            start=(cci == 0),
                                            stop=(cci == 1))
                                    _copy(nc, f,
                                          osts[mc][:, fl * 2048
                                                   + f * 512:
                                                   fl * 2048
                                                   + (f + 1) * 512],
                                          ps5[:])
                        for mc in range(2):
                            nc.scalar.dma_start(
                                out=o[mc, :,
                                      fb * 4096:(fb + 1) * 4096],
                                in_=osts[mc][:])
                sc5.__exit__(None, None, None)
    nc.compile()
    return nc


def _host_prep(P_real, P_imag, V, W1, W2, mixer_real, mixer_imag):
    from ml_dtypes import bfloat16 as bf16
    P_real = np.asarray(P_real, np.float32)
    P_imag = np.asarray(P_imag, np.float32)
    V = np.asarray(V, np.float32)
    W1 = np.asarray(W1, np.float32) * W1_SCALE
    W2 = np.asarray(W2, np.float32) * W2_SCALE
    mr = np.asarray(mixer_real, np.float32)
    mi = np.asarray(mixer_imag, np.float32)

    pt_all = np.stack([P_real.transpose(0, 2, 1),
                       P_imag.transpose(0, 2, 1)], axis=1).astype(bf16)
    perm = np.concatenate(
        [np.r_[32 * c:32 * c + 32, 256 + 32 * c:256 + 32 * c + 32]
         for c in range(NCORES)])
    v_all = np.ascontiguousarray(V[:, :, perm]).astype(bf16)
    w1t = np.ascontiguousarray(W1.reshape(H * dproj, D).T).astype(bf16)
    w2t = np.ascontiguousarray(W2.reshape(H * dproj, D).T).astype(bf16)

    t_idx, k_idx = np.arange(T), np.arange(K)
    ang = 2 * np.pi * np.outer(k_idx, t_idx) / T
    scale = np.where(k_idx[:, None] == 0, 1.0, 2.0) / T
    cr_k = np.cos(ang) * scale
    ci_k = -np.sin(ang) * scale
    # contraction row order p = src*16 + kl*8 + r4*2 + ri, k = 2*src + kl
    cirbm = np.zeros((128, 128), np.float32)
    for p in range(128):
        src, rem = p // 16, p % 16
        kl_, r4, ri_ = rem // 8, (rem % 8) // 2, rem % 2
        k_ = 2 * src + kl_
        coef = cr_k[k_] if ri_ == 0 else ci_k[k_]
        cirbm[p, r4 * 32:r4 * 32 + T] = coef
    cirbm = cirbm.astype(bf16)

    cos2, sin2 = np.cos(ang), np.sin(ang)  # (K, T)
    G = np.empty((H, T, 2, K, H), np.float32)
    G[:, :, 0] = (np.einsum('kt,ij->itkj', cos2, mr)
                  + np.einsum('kt,ij->itkj', sin2, mi))
    G[:, :, 1] = (np.einsum('kt,ij->itkj', cos2, mi)
                  - np.einsum('kt,ij->itkj', sin2, mr))
    G *= G_SCALE
    gm124 = G.reshape(2, 124, 256)
    gm = np.zeros((2, 128, 256), np.float32)
    for h4 in range(4):
        gm[:, h4 * 32:h4 * 32 + 31, :] = gm124[:, h4 * 31:(h4 + 1) * 31, :]
    gm = np.ascontiguousarray(gm).astype(bf16)

    in_maps = []
    for c in range(NCORES):
        in_maps.append({
            "pt": np.ascontiguousarray(pt_all[2 * c:2 * c + 2]),
            "v": np.ascontiguousarray(v_all[2 * c:2 * c + 2]),
            "w1t": w1t, "w2t": w2t, "cirb": cirbm, "g": gm,
        })
    return in_maps


def _assemble(outs):
    res = np.empty((K, R, R, H), np.complex64)
    # rv axes: (k, stripR, cR, swR, stripS, cS, swS, j);
    # R axis (first) <- s~ = (gi, strip', sw'); S axis <- l = (q, blk, sw)
    rv = res.reshape(K, 2, 8, 32, 2, 8, 32, H)
    for c in range(NCORES):
        oc = np.asarray(outs[c], dtype=np.float32)  # (2, 128, 32768)
        arr = (oc[0] + 1j * oc[1]).astype(np.complex64)
        # (k, j, q, blk, sw, gi, strip', sw')
        arr = arr.reshape(K, H, 2, 2, 32, 4, 2, 32)
        gbase = 4 * (c // 4)
        for q in range(2):
            sc = c if q == 0 else c ^ 4
            rv[:, :, gbase:gbase + 4, :, :, sc, :, :] = \
                arr[:, :, q].transpose(0, 5, 4, 6, 2, 3, 1)
    return res


def _enable_axon_trace():
    import types
    if "antenv.axon_hooks" not in sys.modules:
        m = types.ModuleType("antenv.axon_hooks")
        m._hook = None
        m.set_axon_ntff_profile_hook = lambda h: setattr(m, "_hook", h)
        m.get_axon_ntff_profile_hook = lambda: m._hook
        sys.modules["antenv.axon_hooks"] = m
        import antenv
        antenv.axon_hooks = m
        from trn_agent_boot.trn_boot import _ntff_profile_via_ctypes
        hook = _ntff_profile_via_ctypes("/opt/axon/libaxon_pjrt.so")
        m._hook = hook
    bass_utils.upload_artifacts = lambda tmpdir: f"local:{tmpdir}"


def kernel(P_real, P_imag, V, W1, W2, mixer_real, mixer_imag):
    if "nc" not in _CACHE:
        _CACHE["nc"] = _build()
    nc = _CACHE["nc"]
    in_maps = _host_prep(P_real, P_imag, V, W1, W2, mixer_real, mixer_imag)

    if os.environ.get("KSIM"):
        from concourse.bass_interp import MultiCoreSim
        sim = MultiCoreSim(nc, num_cores=NCORES, num_workers=NCORES)
        for c in range(NCORES):
            for k_, arr in in_maps[c].items():
                sim.cores[c].tensor(k_)[:] = arr
        sim.simulate(check_with_hw=False)
        outs = [np.array(sim.cores[c].tensor("o")) for c in range(NCORES)]
        return _assemble(outs)

    trace = bool(os.environ.get("KTRACE"))
    if trace:
        _enable_axon_trace()
    res = bass_utils.run_bass_kernel_spmd(
        nc, in_maps, core_ids=list(range(NCORES)), trace=trace,
        tmpdir=os.environ.get("KTRACE_DIR") or None)
    if trace:
        print(f"HW exec time: {res.exec_time_ns} ns")
        _CACHE["exec_time_ns"] = res.exec_time_ns
        _CACHE["results"] = res
    outs = [res.results[c]["o"] for c in range(NCORES)]
    return _assemble(outs)
